# revision 26
# baseline (speedup 1.0000x reference)
"""Distributed cosine-attention kernel for TRN2 (8 NeuronCores).

Problem (nn_Attention): B=4, N=2048, D_MODEL=1024, HEADS=16, DIM_HEAD=64
  qkv = x @ w_qkv.T + b_qkv ; q,k l2-normalized over head dim;
  attn = softmax(clip-scale * qn @ kn^T); out = (attn @ v) @ w_out.T

Sharding: core c handles batch b=c//2 and global heads hg*8..hg*8+8 (hg=c%2).
Each core computes a partial out^T [D_MODEL, N]; the host sums the two cores
of each batch and transposes.

Per-core dataflow (no on-device transposes needed anywhere):
  - host passes x[b].T ("xT" [C,T]) and pre-transposed weight shards
  - QK proj -> Q^T/K^T [d-on-partition, tok-free], head pairs packed 64+64
  - V proj -> V [tok-on-partition, d-free]; bias via K=1 matmul
  - l2norm: sum of squares per head via mask matmul; rsqrt via Ln+Exp on
    ScalarE (single activation-table set); broadcast across partitions via
    step-0 DMA; logit scale folded into K^T
  - S^T tiles [keys, queries] via f32r row-packed matmuls (two K=64 heads
    in row groups 0/64); P^T = Exp(S^T - s) over 4-bank PSUM groups (bf16)
  - O^T = P@V and the softmax denominator via bf16 col-packed matmuls
    (tile_position (0,0)/(0,64)), PSUM-accumulated over all key tiles
  - out^T partial = woutT tiles @ O (bf16)
All emission is software-pipelined: chain ops for unit u-1 are emitted after
the bulk matmuls of unit u, so the in-order PE stream never stalls on
ACT/DVE round-trips.
"""
import sys
sys.path.insert(0, "/opt/trn_rl_repo")

from dataclasses import dataclass

import numpy as np

try:
    import ml_dtypes
    ml_bf16 = ml_dtypes.bfloat16
except ImportError:  # pragma: no cover
    ml_bf16 = np.float32

import concourse.bass as bass
import concourse.tile as tile
import concourse.mybir as mybir
from concourse import bacc
from concourse.bass_utils import run_bass_kernel_spmd

F32 = mybir.dt.float32
F32R = mybir.dt.float32r
BF16 = mybir.dt.bfloat16
AF = mybir.ActivationFunctionType

D_MODEL = 1024
HEADS = 16
DIM_HEAD = 64
INNER = HEADS * DIM_HEAD
B = 4
N = 2048
N_CORES = 8
LOG100 = float(np.log(100.0))

_ACT_SET = "natural_log_exp_and_others"
_tables_patched = False


def _patch_act_tables():
    """Make every activation resolve to one table set (it contains ln, exp,
    square, copy, identity) so no ACT_TABLE_LOAD thrash occurs."""
    global _tables_patched
    if _tables_patched:
        return
    orig = bacc.get_activation_tables

    def patched(arch):
        tabs = orig(arch)
        if _ACT_SET in tabs:
            tabs = {k: (v if k == _ACT_SET else set())
                    for k, v in tabs.items()}
        return tabs

    bacc.get_activation_tables = patched
    _tables_patched = True


@dataclass
class Cfg:
    T: int = N
    C: int = D_MODEL
    NH: int = 8
    DH: int = DIM_HEAD
    QB: int = 512
    SG: int = 1             # k-tiles per exp group
    merge_pairs: tuple = (True, True, True, True)

    @property
    def PAIRS(self):
        return self.NH // 2

    @property
    def CT(self):
        return self.C // 128

    @property
    def KT(self):
        return self.T // 128

    @property
    def NQB(self):
        return self.T // self.QB

    @property
    def VW(self):
        return self.NH * self.DH


def build(cfg: Cfg):
    _patch_act_tables()
    T, C, QB = cfg.T, cfg.C, cfg.QB
    PAIRS, CT, KT, NQB, VW = cfg.PAIRS, cfg.CT, cfg.KT, cfg.NQB, cfg.VW
    SG = cfg.SG

    nc = bacc.Bacc("TRN2", target_bir_lowering=False, debug=False,
                   enable_asserts=False)

    xT_d = nc.declare_dram_parameter("xT", [C, T], BF16, isOutput=False)
    wqkT_d = nc.declare_dram_parameter("wqkT", [C, 2 * PAIRS * 128], BF16,
                                       isOutput=False)
    bqk_d = nc.declare_dram_parameter("bqk", [2 * PAIRS, 128, 1], F32,
                                      isOutput=False)
    wvT_d = nc.declare_dram_parameter("wvT", [C, VW], BF16, isOutput=False)
    # out-proj weights, head-half major: [64, (2p+h), C]
    woT_d = nc.declare_dram_parameter("woT", [64, 2 * PAIRS * C], BF16,
                                      isOutput=False)
    # per-head scale constants: [:, 0]=-s_h (exp bias), [:, 1]=ln(s_h)
    scl_d = nc.declare_dram_parameter("scl", [cfg.NH, 2, 1], F32,
                                      isOutput=False)
    sel2T_d = nc.declare_dram_parameter("sel2T", [128, 2], F32,
                                        isOutput=False)
    onec_d = nc.declare_dram_parameter("onec", [1, 1], BF16, isOutput=False)
    # out-proj bias columns (wo @ bv for this shard), per 128-row block
    wob_d = nc.declare_dram_parameter("wob", [CT, 128, 1], F32,
                                      isOutput=False)
    out_d = nc.declare_dram_parameter("out", [C, T], BF16, isOutput=True)

    with tile.TileContext(nc) as tc:
        with (
            tc.tile_pool(name="const", bufs=1) as const,
            tc.tile_pool(name="dram", bufs=1, space="DRAM") as dram,
            tc.tile_pool(name="xt", bufs=1) as xt_pool,
            tc.tile_pool(name="qksb", bufs=2) as qk_sb,
            tc.tile_pool(name="norm", bufs=2) as norm_sb,
            tc.tile_pool(name="vtmp", bufs=2) as vtmp_pool,
            tc.tile_pool(name="att", bufs=2) as att_sb,
            tc.tile_pool(name="pt", bufs=2) as pt_pool,
            tc.tile_pool(name="ofin", bufs=1) as ofin_pool,
            tc.tile_pool(name="otout", bufs=2) as ot_pool,
            # PSUM budget (8 banks): mm 2 + sg 2x2 + pv 2 (fused denom)
            tc.tile_pool(name="psmm", bufs=2, space="PSUM") as ps_mm,
            tc.tile_pool(name="pssg", bufs=2, space="PSUM") as ps_sg,
            tc.tile_pool(name="pspv", bufs=1, space="PSUM") as ps_pv,
        ):
            # ---- DRAM spill tensors ----
            qhat_sp = [dram.tile([128, T], BF16, tag=f"qsp{p}",
                                 name=f"qsp{p}") for p in range(PAIRS)]
            khat_sp = [dram.tile([128, T], BF16, tag=f"ksp{p}",
                                 name=f"ksp{p}") for p in range(PAIRS)]
            vhat_sp = dram.tile([KT, 128, VW], BF16, tag="vsp")
            rq_sp = [dram.tile([2, QB], F32, tag=f"rqsp{i}", name=f"rqsp{i}")
                     for i in range(2)]
            rl_sp = [dram.tile([1, 2 * QB], F32, tag=f"rlsp{i}",
                               name=f"rlsp{i}") for i in range(2)]

            # ---- constants ----
            sel2T_f = const.tile([128, 2], F32, tag="sel2Tf")
            nc.sync.dma_start(out=sel2T_f, in_=sel2T_d.ap())
            sel2T = const.tile([128, 2], F32R, tag="sel2T")
            nc.vector.tensor_copy(sel2T, sel2T_f)

            nbias_cols = []
            for h in range(cfg.NH):
                col = const.tile([128, 1], F32, tag=f"nb{h}", name=f"nb{h}")
                nc.sync.dma_start(
                    out=col, in_=scl_d.ap()[h, 0:1, :].to_broadcast((128, 1)))
                nbias_cols.append(col)
            lns_cols = []
            for p in range(PAIRS):
                col = const.tile([2, 1], F32, tag=f"lns{p}", name=f"lns{p}")
                nc.sync.dma_start(out=col,
                                  in_=scl_d.ap()[2 * p:2 * p + 2, 1, :])
                lns_cols.append(col)
            zero_col = const.tile([2, 1], F32, tag="zeroc")
            nc.vector.memset(zero_col, 0.0)

            bqk_cols = []
            for it in range(2 * PAIRS):
                col = const.tile([128, 1], F32, tag=f"bqk{it}",
                                 name=f"bqk{it}")
                nc.sync.dma_start(out=col, in_=bqk_d.ap()[it])
                bqk_cols.append(col)
            wob_cols = []
            for cb in range(CT):
                col = const.tile([128, 1], F32, tag=f"wob{cb}",
                                 name=f"wob{cb}")
                nc.sync.dma_start(out=col, in_=wob_d.ap()[cb])
                wob_cols.append(col)

            wqk_all = const.tile([128, CT, 2 * PAIRS * 128], BF16,
                                 tag="wqk_all")
            nc.sync.dma_start(
                out=wqk_all,
                in_=wqkT_d.ap().rearrange("(ct p) i -> p ct i", p=128))
            wv_res = const.tile([128, CT, VW], BF16, tag="wv_res")
            nc.sync.dma_start(
                out=wv_res,
                in_=wvT_d.ap().rearrange("(ct p) v -> p ct v", p=128))
            wo_res = const.tile([64, 2 * PAIRS, C], BF16, tag="wo_res")
            nc.sync.dma_start(
                out=wo_res,
                in_=woT_d.ap().rearrange("d (ph c) -> d ph c",
                                         ph=2 * PAIRS))

            xt = []
            for ct in range(CT):
                t = xt_pool.tile([128, T], BF16, tag=f"xt{ct}",
                                 name=f"xt{ct}")
                nc.sync.dma_start(out=t,
                                  in_=xT_d.ap()[ct * 128:(ct + 1) * 128, :])
                xt.append(t)

            # ================= V projection (pipelined evac) ==============
            pend_v = None

            def flush_v():
                nonlocal pend_v
                if pend_v is None:
                    return
                tt, vps = pend_v
                vtmp = vtmp_pool.tile([128, VW], BF16, tag="vtmp",
                                      name=f"vtmp{tt}")
                nc.vector.tensor_copy(vtmp, vps)
                nc.sync.dma_start(out=vhat_sp[tt], in_=vtmp)
                pend_v = None

            for tt in range(KT):
                vps = ps_mm.tile([128, VW], F32, tag="mm", name=f"vps{tt}")
                for ct in range(CT):
                    nc.tensor.matmul(vps, xt[ct][:, tt * 128:(tt + 1) * 128],
                                     wv_res[:, ct, :], start=(ct == 0),
                                     stop=(ct == CT - 1))
                flush_v()
                pend_v = (tt, vps)
            flush_v()

            # ============ QK projection + l2norm (pipelined) ============
            pend_qk = None

            def flush_qk():
                nonlocal pend_qk
                if pend_qk is None:
                    return
                p, is_k, tb, it, qs = pend_qk
                ts = slice(tb * QB, (tb + 1) * QB)
                uid = f"{it}_{tb}"
                qraw = qk_sb.tile([128, QB], F32, tag="qraw",
                                  name=f"qraw{uid}")
                nc.vector.tensor_scalar_add(qraw, qs, bqk_cols[it])
                q2 = qk_sb.tile([128, QB], F32R, tag="q2", name=f"q2{uid}")
                nc.vector.tensor_mul(q2, qraw, qraw)
                ss = ps_mm.tile([2, QB], F32, tag="mm", name=f"ss{uid}")
                nc.tensor.matmul(ss, sel2T[:], q2[:], start=True, stop=True)
                lss = norm_sb.tile([2, QB], F32, tag="lss", name=f"lss{uid}")
                nc.scalar.activation(lss, ss, AF.Ln)
                rq = norm_sb.tile([2, QB], F32, tag="rq", name=f"rq{uid}")
                nc.scalar.activation(rq, lss, AF.Exp, scale=-0.5,
                                     bias=lns_cols[p] if is_k
                                     else zero_col[:])
                rqd = rq_sp[(2 * tb + it) % 2]
                nc.sync.dma_start(out=rqd, in_=rq)
                rqbc = qk_sb.tile([128, QB], F32, tag="rqbc",
                                  name=f"rqbc{uid}")
                nc.sync.dma_start(out=rqbc[0:64, :],
                                  in_=rqd[0:1, :].to_broadcast((64, QB)))
                nc.sync.dma_start(out=rqbc[64:128, :],
                                  in_=rqd[1:2, :].to_broadcast((64, QB)))
                qhat = qk_sb.tile([128, QB], BF16, tag="qhat",
                                  name=f"qhat{uid}")
                nc.vector.tensor_mul(qhat, qraw, rqbc)
                dst = khat_sp[p] if is_k else qhat_sp[p]
                nc.sync.dma_start(out=dst[:, ts], in_=qhat)
                pend_qk = None

            def emit_proj_pair(p):
                nonlocal pend_qk
                for is_k in (0, 1):
                    it = 2 * p + is_k
                    for tb in range(NQB):
                        ts = slice(tb * QB, (tb + 1) * QB)
                        qs = ps_mm.tile([128, QB], F32, tag="mm",
                                        name=f"qs{it}_{tb}")
                        for ct in range(CT):
                            nc.tensor.matmul(
                                qs,
                                wqk_all[:, ct, it * 128:(it + 1) * 128],
                                xt[ct][:, ts], start=(ct == 0),
                                stop=(ct == CT - 1))
                        flush_qk()
                        pend_qk = (p, is_k, tb, it, qs)

            # ================= attention (pipelined) =================
            o_fin = {}
            NSG = KT // SG

            def emit_att_pair(p, qb_done=None):
                kk = att_sb.tile([128, T], BF16, tag="kk", name=f"kk{p}")
                nc.sync.dma_start(out=kk, in_=khat_sp[p])
                qq = att_sb.tile([128, T], BF16, tag="qq", name=f"qq{p}")
                nc.sync.dma_start(out=qq, in_=qhat_sp[p])
                # V with a fused ones column per head: [128, KT, 2, 65]
                vv = att_sb.tile([128, KT, 2, 65], BF16, tag="vv",
                                 name=f"vv{p}")
                for h in (0, 1):
                    nc.sync.dma_start(
                        out=vv[:, :, h, 64:65],
                        in_=onec_d.ap().to_broadcast((128, KT, 1)))
                for h in (0, 1):
                    nc.sync.dma_start(
                        out=vv[:, :, h, 0:64],
                        in_=vhat_sp[:, :, p * 128 + h * 64:
                                    p * 128 + (h + 1) * 64].rearrange(
                            "kt pp w -> pp kt w"))
                for qb in range(NQB):
                    qsl = slice(qb * QB, (qb + 1) * QB)
                    pv = ps_pv.tile([128, 2, QB], F32, tag="pv",
                                    name=f"pv{p}_{qb}")

                    def emit_pvlb(g, ptile, pv=pv, vv=vv):
                        for j in range(SG):
                            kt = g * SG + j
                            first = kt == 0
                            last = kt == KT - 1
                            nc.tensor.matmul(pv[0:65, 0, :], vv[:, kt, 0, :],
                                             ptile[:, 0, j, :], start=first,
                                             stop=last)
                            nc.tensor.matmul(pv[0:65, 1, :], vv[:, kt, 1, :],
                                             ptile[:, 1, j, :], start=first,
                                             stop=last, skip_group_check=True)

                    pend_att = None
                    for g in range(NSG):
                        sg = ps_sg.tile([128, 2, SG, QB], F32, tag="sg",
                                        name=f"sg{p}_{qb}_{g}")
                        for j in range(SG):
                            kt = g * SG + j
                            ksl = slice(kt * 128, (kt + 1) * 128)
                            nc.tensor.matmul(sg[:, 0, j, :], kk[0:64, ksl],
                                             qq[0:64, qsl], start=True,
                                             stop=True)
                            nc.tensor.matmul(sg[:, 1, j, :], kk[64:128, ksl],
                                             qq[64:128, qsl], start=True,
                                             stop=True)
                        ptile = pt_pool.tile([128, 2, SG, QB], BF16, tag="pt",
                                             name=f"pt{p}_{qb}_{g}")
                        if cfg.merge_pairs[p]:
                            nc.scalar.activation(ptile, sg, AF.Exp,
                                                 bias=nbias_cols[2 * p][:])
                        else:
                            nc.scalar.activation(ptile[:, 0], sg[:, 0],
                                                 AF.Exp,
                                                 bias=nbias_cols[2 * p][:])
                            nc.scalar.activation(
                                ptile[:, 1], sg[:, 1], AF.Exp,
                                bias=nbias_cols[2 * p + 1][:])
                        if pend_att is not None:
                            emit_pvlb(*pend_att)
                        pend_att = (g, ptile)
                    emit_pvlb(*pend_att)

                    rl = att_sb.tile([128, 2, QB], F32, tag="rl",
                                     name=f"rl{p}_{qb}")
                    nc.vector.reciprocal_approx_fast(out=rl[0:65, 0, :],
                                                     in_=pv[0:65, 0, :])
                    nc.vector.reciprocal_approx_fast(out=rl[0:65, 1, :],
                                                     in_=pv[0:65, 1, :])
                    rld = rl_sp[qb % 2]
                    nc.gpsimd.dma_start(out=rld, in_=rl[64:65, :, :])
                    rlbc = att_sb.tile([64, 2, QB], F32, tag="rlbc",
                                       name=f"rlbc{p}_{qb}")
                    for h in (0, 1):
                        nc.gpsimd.dma_start(
                            out=rlbc[:, h, :],
                            in_=rld[0:1, h * QB:(h + 1) * QB].to_broadcast(
                                (64, QB)))
                    for h in (0, 1):
                        of = ofin_pool.tile([64, QB], BF16,
                                            tag=f"of{p}_{h}_{qb}",
                                            name=f"of{p}_{h}_{qb}")
                        nc.vector.tensor_mul(of, pv[0:64, h, :],
                                             rlbc[:, h, :])
                        o_fin[(p, h, qb)] = of
                    if qb_done is not None:
                        qb_done(qb)

            # ================= out projection (pipelined evac) ============
            pend_o = None
            outT_v = out_d.ap().rearrange("(cb p) t -> p cb t", p=128)

            def flush_o():
                nonlocal pend_o
                if pend_o is None:
                    return
                qb, cb, op, ot = pend_o
                nc.vector.tensor_scalar_add(ot[:, cb, :], op, wob_cols[cb])
                pend_o = None

            def emit_outproj_qb(qb):
                nonlocal pend_o
                ot = ot_pool.tile([128, CT, QB], BF16, tag="ot",
                                  name=f"ot{qb}")
                for cb in range(CT):
                    op = ps_mm.tile([128, QB], F32, tag="mm",
                                    name=f"op{qb}_{cb}")
                    for ph in range(2 * PAIRS):
                        nc.tensor.matmul(op,
                                         wo_res[:, ph,
                                                cb * 128:(cb + 1) * 128],
                                         o_fin[(ph // 2, ph % 2, qb)][:],
                                         start=(ph == 0),
                                         stop=(ph == 2 * PAIRS - 1))
                    flush_o()
                    pend_o = (qb, cb, op, ot)
                flush_o()
                nc.gpsimd.dma_start(
                    out=outT_v[:, :, qb * QB:(qb + 1) * QB], in_=ot)

            # ======== interleaved pair-level schedule ========
            for p in range(PAIRS):
                emit_proj_pair(p)
            flush_qk()
            for p in range(PAIRS - 1):
                emit_att_pair(p)
            emit_att_pair(PAIRS - 1, qb_done=emit_outproj_qb)
            flush_o()

    nc.compile()
    return nc


# ======================= host-side sharding =======================

def shard_inputs(x, w_qkv, b_qkv, w_out, logit_scale):
    x = np.ascontiguousarray(np.asarray(x, dtype=np.float32))
    w_qkv = np.asarray(w_qkv, dtype=np.float32)
    b_qkv = np.asarray(b_qkv, dtype=np.float32)
    w_out = np.asarray(w_out, dtype=np.float32)
    ls = np.asarray(logit_scale, dtype=np.float32).reshape(-1)
    s_all = np.exp(np.minimum(ls, LOG100)).astype(np.float32)

    Wq = w_qkv[0 * INNER:1 * INNER]
    Wk = w_qkv[1 * INNER:2 * INNER]
    Wv = w_qkv[2 * INNER:3 * INNER]
    bq = b_qkv[0 * INNER:1 * INNER]
    bk = b_qkv[1 * INNER:2 * INNER]
    bv = b_qkv[2 * INNER:3 * INNER]

    xT = [np.ascontiguousarray(x[b].T.astype(ml_bf16)) for b in range(B)]

    per_hg = {}
    merge = [True] * 4
    for hg in range(2):
        heads = list(range(hg * 8, hg * 8 + 8))
        rows, brows = [], []
        for p in range(4):
            g0, g1 = heads[2 * p], heads[2 * p + 1]
            if s_all[g0] != s_all[g1]:
                merge[p] = False
            rows += [Wq[g0 * 64:(g0 + 1) * 64], Wq[g1 * 64:(g1 + 1) * 64],
                     Wk[g0 * 64:(g0 + 1) * 64], Wk[g1 * 64:(g1 + 1) * 64]]
            brows += [bq[g0 * 64:(g0 + 1) * 64], bq[g1 * 64:(g1 + 1) * 64],
                      bk[g0 * 64:(g0 + 1) * 64], bk[g1 * 64:(g1 + 1) * 64]]
        wqkT = np.ascontiguousarray(
            np.concatenate(rows, axis=0).T.astype(ml_bf16))
        bqk = np.ascontiguousarray(
            np.concatenate(brows, axis=0)).reshape(8, 128, 1)
        vsl = slice(hg * 512, (hg + 1) * 512)
        wvT = np.ascontiguousarray(Wv[vsl].T.astype(ml_bf16))
        # [64, (2p+h)*C]: head-half major out-proj weights for K=64 matmuls
        woT = np.ascontiguousarray(
            w_out[:, vsl].reshape(D_MODEL, 4, 2, 64).transpose(3, 1, 2, 0)
            .reshape(64, 8 * D_MODEL).astype(ml_bf16))
        wob = np.ascontiguousarray(
            (w_out[:, vsl] @ bv[vsl]).astype(np.float32)).reshape(8, 128, 1)
        scl = np.stack([-s_all[heads], np.log(s_all[heads])],
                       axis=1).astype(np.float32).reshape(8, 2, 1)
        per_hg[hg] = dict(wqkT=wqkT, bqk=bqk, wvT=wvT, woT=woT, wob=wob,
                          scl=scl)

    sel2 = np.zeros((2, 128), dtype=np.float32)
    sel2[0, 0:64] = 1.0
    sel2[1, 64:128] = 1.0
    sel2T = np.ascontiguousarray(sel2.T)
    onec = np.ones((1, 1), dtype=ml_bf16)
    in_maps = []
    for c in range(N_CORES):
        b, hg = c // 2, c % 2
        m = dict(per_hg[hg])
        m["xT"] = xT[b]
        m["sel2T"] = sel2T
        m["onec"] = onec
        in_maps.append(m)
    return in_maps, tuple(merge)


_NC_CACHE = {}
TRACE = False
LAST_RESULT = None


def kernel(x, w_qkv, b_qkv, w_out, logit_scale):
    global LAST_RESULT
    in_maps, merge_pairs = shard_inputs(x, w_qkv, b_qkv, w_out, logit_scale)
    cfg = Cfg(merge_pairs=merge_pairs)
    if merge_pairs not in _NC_CACHE:
        _NC_CACHE[merge_pairs] = build(cfg)
    nc = _NC_CACHE[merge_pairs]
    res = run_bass_kernel_spmd(nc, in_maps, core_ids=list(range(N_CORES)),
                               trace=TRACE)
    LAST_RESULT = res
    outs = [np.asarray(res.results[c]["out"], dtype=np.float32)
            for c in range(N_CORES)]
    full = np.empty((B, N, D_MODEL), dtype=np.float32)
    for b in range(B):
        full[b] = (outs[2 * b] + outs[2 * b + 1]).T
    return full



# revision 27
# speedup vs baseline: 1.1919x; 1.1919x over previous
"""Distributed cosine-attention kernel for TRN2 (8 NeuronCores).

Problem (nn_Attention): B=4, N=2048, D_MODEL=1024, HEADS=16, DIM_HEAD=64
  qkv = x @ w_qkv.T + b_qkv ; q,k l2-normalized over head dim;
  attn = softmax(clip-scale * qn @ kn^T); out = (attn @ v) @ w_out.T

Sharding: core c handles batch b=c//2 and global heads hg*8..hg*8+8 (hg=c%2).
Each core computes a partial out^T [D_MODEL, N]; the host sums the two cores
of each batch and transposes.

Per-core dataflow (no on-device transposes needed anywhere):
  - host passes x[b].T ("xT" [C,T]) and pre-transposed weight shards
  - QK proj -> Q^T/K^T [d-on-partition, tok-free], head pairs packed 64+64
  - V proj -> V [tok-on-partition, d-free]; bias via K=1 matmul
  - l2norm: sum of squares per head via mask matmul; rsqrt via Ln+Exp on
    ScalarE (single activation-table set); broadcast across partitions via
    step-0 DMA; logit scale folded into K^T
  - S^T tiles [keys, queries] via f32r row-packed matmuls (two K=64 heads
    in row groups 0/64); P^T = Exp(S^T - s) over 4-bank PSUM groups (bf16)
  - O^T = P@V and the softmax denominator via bf16 col-packed matmuls
    (tile_position (0,0)/(0,64)), PSUM-accumulated over all key tiles
  - out^T partial = woutT tiles @ O (bf16)
All emission is software-pipelined: chain ops for unit u-1 are emitted after
the bulk matmuls of unit u, so the in-order PE stream never stalls on
ACT/DVE round-trips.
"""
import sys
sys.path.insert(0, "/opt/trn_rl_repo")

from dataclasses import dataclass

import numpy as np

try:
    import ml_dtypes
    ml_bf16 = ml_dtypes.bfloat16
except ImportError:  # pragma: no cover
    ml_bf16 = np.float32

import concourse.bass as bass
import concourse.tile as tile
import concourse.mybir as mybir
from concourse import bacc
from concourse.bass_utils import run_bass_kernel_spmd

F32 = mybir.dt.float32
F32R = mybir.dt.float32r
BF16 = mybir.dt.bfloat16
AF = mybir.ActivationFunctionType

D_MODEL = 1024
HEADS = 16
DIM_HEAD = 64
INNER = HEADS * DIM_HEAD
B = 4
N = 2048
N_CORES = 8
LOG100 = float(np.log(100.0))

_ACT_SET = "natural_log_exp_and_others"
_tables_patched = False


def _patch_act_tables():
    """Make every activation resolve to one table set (it contains ln, exp,
    square, copy, identity) so no ACT_TABLE_LOAD thrash occurs."""
    global _tables_patched
    if _tables_patched:
        return
    orig = bacc.get_activation_tables

    def patched(arch):
        tabs = orig(arch)
        if _ACT_SET in tabs:
            tabs = {k: (v if k == _ACT_SET else set())
                    for k, v in tabs.items()}
        return tabs

    bacc.get_activation_tables = patched
    _tables_patched = True


@dataclass
class Cfg:
    T: int = N
    C: int = D_MODEL
    NH: int = 8
    DH: int = DIM_HEAD
    QB: int = 512
    SG: int = 1             # k-tiles per exp group
    merge_pairs: tuple = (True, True, True, True)

    @property
    def PAIRS(self):
        return self.NH // 2

    @property
    def CT(self):
        return self.C // 128

    @property
    def KT(self):
        return self.T // 128

    @property
    def NQB(self):
        return self.T // self.QB

    @property
    def VW(self):
        return self.NH * self.DH


def build(cfg: Cfg):
    _patch_act_tables()
    T, C, QB = cfg.T, cfg.C, cfg.QB
    PAIRS, CT, KT, NQB, VW = cfg.PAIRS, cfg.CT, cfg.KT, cfg.NQB, cfg.VW
    SG = cfg.SG

    nc = bacc.Bacc("TRN2", target_bir_lowering=False, debug=False,
                   enable_asserts=False)

    xT_d = nc.declare_dram_parameter("xT", [C, T], BF16, isOutput=False)
    wqkT_d = nc.declare_dram_parameter("wqkT", [C, 2 * PAIRS * 128], BF16,
                                       isOutput=False)
    bqk_d = nc.declare_dram_parameter("bqk", [2 * PAIRS, 128, 1], F32,
                                      isOutput=False)
    wvT_d = nc.declare_dram_parameter("wvT", [C, VW], BF16, isOutput=False)
    # out-proj weights, head-half major: [64, (2p+h), C]
    woT_d = nc.declare_dram_parameter("woT", [64, 2 * PAIRS * C], BF16,
                                      isOutput=False)
    # per-head scale constants: [:, 0]=-s_h (exp bias), [:, 1]=ln(s_h)
    scl_d = nc.declare_dram_parameter("scl", [cfg.NH, 2, 1], F32,
                                      isOutput=False)
    sel2T_d = nc.declare_dram_parameter("sel2T", [128, 2], F32,
                                        isOutput=False)
    onec_d = nc.declare_dram_parameter("onec", [1, 1], BF16, isOutput=False)
    # out-proj bias columns (wo @ bv for this shard), per 128-row block
    wob_d = nc.declare_dram_parameter("wob", [CT, 128, 1], F32,
                                      isOutput=False)
    out_d = nc.declare_dram_parameter("out", [C, T], BF16, isOutput=True)

    with tile.TileContext(nc) as tc:
        with (
            tc.tile_pool(name="const", bufs=1) as const,
            tc.tile_pool(name="dram", bufs=1, space="DRAM") as dram,
            tc.tile_pool(name="xt", bufs=1) as xt_pool,
            tc.tile_pool(name="qksb", bufs=2) as qk_sb,
            tc.tile_pool(name="norm", bufs=2) as norm_sb,
            tc.tile_pool(name="vtmp", bufs=2) as vtmp_pool,
            tc.tile_pool(name="att", bufs=2) as att_sb,
            tc.tile_pool(name="pt", bufs=2) as pt_pool,
            tc.tile_pool(name="ofin", bufs=1) as ofin_pool,
            tc.tile_pool(name="otout", bufs=2) as ot_pool,
            # PSUM budget (8 banks): mm 2 + sg 2x2 + pv 2 (fused denom)
            tc.tile_pool(name="psmm", bufs=2, space="PSUM") as ps_mm,
            tc.tile_pool(name="pssg", bufs=2, space="PSUM") as ps_sg,
            tc.tile_pool(name="pspv", bufs=1, space="PSUM") as ps_pv,
        ):
            # ---- DRAM spill tensors ----
            qhat_sp = [dram.tile([128, T], BF16, tag=f"qsp{p}",
                                 name=f"qsp{p}") for p in range(PAIRS)]
            khat_sp = [dram.tile([128, T], BF16, tag=f"ksp{p}",
                                 name=f"ksp{p}") for p in range(PAIRS)]
            rq_sp = [dram.tile([2, QB], F32, tag=f"rqsp{i}", name=f"rqsp{i}")
                     for i in range(2)]
            rl_sp = [dram.tile([1, 2 * QB], F32, tag=f"rlsp{i}",
                               name=f"rlsp{i}") for i in range(2)]

            # ---- constants ----
            sel2T_f = const.tile([128, 2], F32, tag="sel2Tf")
            nc.sync.dma_start(out=sel2T_f, in_=sel2T_d.ap())
            sel2T = const.tile([128, 2], F32R, tag="sel2T")
            nc.vector.tensor_copy(sel2T, sel2T_f)

            nbias_cols = []
            for h in range(cfg.NH):
                col = const.tile([128, 1], F32, tag=f"nb{h}", name=f"nb{h}")
                nc.sync.dma_start(
                    out=col, in_=scl_d.ap()[h, 0:1, :].to_broadcast((128, 1)))
                nbias_cols.append(col)
            lns_cols = []
            for p in range(PAIRS):
                col = const.tile([2, 1], F32, tag=f"lns{p}", name=f"lns{p}")
                nc.sync.dma_start(out=col,
                                  in_=scl_d.ap()[2 * p:2 * p + 2, 1, :])
                lns_cols.append(col)
            zero_col = const.tile([2, 1], F32, tag="zeroc")
            nc.vector.memset(zero_col, 0.0)

            bqk_cols = []
            for it in range(2 * PAIRS):
                col = const.tile([128, 1], F32, tag=f"bqk{it}",
                                 name=f"bqk{it}")
                nc.sync.dma_start(out=col, in_=bqk_d.ap()[it])
                bqk_cols.append(col)
            wob_cols = []
            for cb in range(CT):
                col = const.tile([128, 1], F32, tag=f"wob{cb}",
                                 name=f"wob{cb}")
                nc.sync.dma_start(out=col, in_=wob_d.ap()[cb])
                wob_cols.append(col)

            wqk_all = const.tile([128, CT, 2 * PAIRS * 128], BF16,
                                 tag="wqk_all")
            nc.sync.dma_start(
                out=wqk_all,
                in_=wqkT_d.ap().rearrange("(ct p) i -> p ct i", p=128))
            wv_res = const.tile([128, CT, VW], BF16, tag="wv_res")
            nc.sync.dma_start(
                out=wv_res,
                in_=wvT_d.ap().rearrange("(ct p) v -> p ct v", p=128))
            wo_res = const.tile([64, 2 * PAIRS, C], BF16, tag="wo_res")
            nc.sync.dma_start(
                out=wo_res,
                in_=woT_d.ap().rearrange("d (ph c) -> d ph c",
                                         ph=2 * PAIRS))

            vall = const.tile([128, KT, 2 * PAIRS, 65], BF16, tag="vall")
            nc.vector.memset(vall, 1.0)

            xt = []
            for ct in range(CT):
                t = xt_pool.tile([128, T], BF16, tag=f"xt{ct}",
                                 name=f"xt{ct}")
                nc.sync.dma_start(out=t,
                                  in_=xT_d.ap()[ct * 128:(ct + 1) * 128, :])
                xt.append(t)

            # ================= V projection (pipelined evac) ==============
            pend_v = None

            def flush_v():
                nonlocal pend_v
                if pend_v is None:
                    return
                tt, vps = pend_v
                nc.vector.tensor_copy(
                    vall[:, tt, :, 0:64],
                    vps.rearrange("p (hh d) -> p hh d", hh=2 * PAIRS))
                pend_v = None

            for tt in range(KT):
                vps = ps_mm.tile([128, VW], F32, tag="mm", name=f"vps{tt}")
                for ct in range(CT):
                    nc.tensor.matmul(vps, xt[ct][:, tt * 128:(tt + 1) * 128],
                                     wv_res[:, ct, :], start=(ct == 0),
                                     stop=(ct == CT - 1))
                flush_v()
                pend_v = (tt, vps)
            flush_v()

            # ============ QK projection + l2norm (pipelined) ============
            pend_qk = None

            def flush_qk():
                nonlocal pend_qk
                if pend_qk is None:
                    return
                p, is_k, tb, it, qs = pend_qk
                ts = slice(tb * QB, (tb + 1) * QB)
                uid = f"{it}_{tb}"
                qraw = qk_sb.tile([128, QB], F32, tag="qraw",
                                  name=f"qraw{uid}")
                nc.vector.tensor_scalar_add(qraw, qs, bqk_cols[it])
                q2 = qk_sb.tile([128, QB], F32R, tag="q2", name=f"q2{uid}")
                nc.vector.tensor_mul(q2, qraw, qraw)
                ss = ps_mm.tile([2, QB], F32, tag="mm", name=f"ss{uid}")
                nc.tensor.matmul(ss, sel2T[:], q2[:], start=True, stop=True)
                lss = norm_sb.tile([2, QB], F32, tag="lss", name=f"lss{uid}")
                nc.scalar.activation(lss, ss, AF.Ln)
                rq = norm_sb.tile([2, QB], F32, tag="rq", name=f"rq{uid}")
                nc.scalar.activation(rq, lss, AF.Exp, scale=-0.5,
                                     bias=lns_cols[p] if is_k
                                     else zero_col[:])
                rqd = rq_sp[(2 * tb + it) % 2]
                nc.sync.dma_start(out=rqd, in_=rq)
                rqbc = qk_sb.tile([128, QB], F32, tag="rqbc",
                                  name=f"rqbc{uid}")
                nc.sync.dma_start(out=rqbc[0:64, :],
                                  in_=rqd[0:1, :].to_broadcast((64, QB)))
                nc.sync.dma_start(out=rqbc[64:128, :],
                                  in_=rqd[1:2, :].to_broadcast((64, QB)))
                qhat = qk_sb.tile([128, QB], BF16, tag="qhat",
                                  name=f"qhat{uid}")
                nc.vector.tensor_mul(qhat, qraw, rqbc)
                dst = khat_sp[p] if is_k else qhat_sp[p]
                nc.sync.dma_start(out=dst[:, ts], in_=qhat)
                pend_qk = None

            def emit_proj_pair(p):
                nonlocal pend_qk
                for is_k in (0, 1):
                    it = 2 * p + is_k
                    for tb in range(NQB):
                        ts = slice(tb * QB, (tb + 1) * QB)
                        qs = ps_mm.tile([128, QB], F32, tag="mm",
                                        name=f"qs{it}_{tb}")
                        for ct in range(CT):
                            nc.tensor.matmul(
                                qs,
                                wqk_all[:, ct, it * 128:(it + 1) * 128],
                                xt[ct][:, ts], start=(ct == 0),
                                stop=(ct == CT - 1))
                        flush_qk()
                        pend_qk = (p, is_k, tb, it, qs)

            # ================= attention (pipelined) =================
            o_fin = {}
            NSG = KT // SG

            def emit_att_pair(p, qb_done=None):
                kk = att_sb.tile([128, T], BF16, tag="kk", name=f"kk{p}")
                nc.sync.dma_start(out=kk, in_=khat_sp[p])
                qq = att_sb.tile([128, T], BF16, tag="qq", name=f"qq{p}")
                nc.sync.dma_start(out=qq, in_=qhat_sp[p])
                for qb in range(NQB):
                    qsl = slice(qb * QB, (qb + 1) * QB)
                    pv = ps_pv.tile([128, 2, QB], F32, tag="pv",
                                    name=f"pv{p}_{qb}")

                    def emit_pvlb(g, ptile, pv=pv):
                        for j in range(SG):
                            kt = g * SG + j
                            first = kt == 0
                            last = kt == KT - 1
                            nc.tensor.matmul(pv[0:65, 0, :],
                                             vall[:, kt, 2 * p, :],
                                             ptile[:, 0, j, :], start=first,
                                             stop=last)
                            nc.tensor.matmul(pv[0:65, 1, :],
                                             vall[:, kt, 2 * p + 1, :],
                                             ptile[:, 1, j, :], start=first,
                                             stop=last, skip_group_check=True)

                    pend_att = None
                    for g in range(NSG):
                        sg = ps_sg.tile([128, 2, SG, QB], F32, tag="sg",
                                        name=f"sg{p}_{qb}_{g}")
                        for j in range(SG):
                            kt = g * SG + j
                            ksl = slice(kt * 128, (kt + 1) * 128)
                            nc.tensor.matmul(sg[:, 0, j, :], kk[0:64, ksl],
                                             qq[0:64, qsl], start=True,
                                             stop=True)
                            nc.tensor.matmul(sg[:, 1, j, :], kk[64:128, ksl],
                                             qq[64:128, qsl], start=True,
                                             stop=True)
                        ptile = pt_pool.tile([128, 2, SG, QB], BF16, tag="pt",
                                             name=f"pt{p}_{qb}_{g}")
                        if cfg.merge_pairs[p]:
                            nc.scalar.activation(ptile, sg, AF.Exp,
                                                 bias=nbias_cols[2 * p][:])
                        else:
                            nc.scalar.activation(ptile[:, 0], sg[:, 0],
                                                 AF.Exp,
                                                 bias=nbias_cols[2 * p][:])
                            nc.scalar.activation(
                                ptile[:, 1], sg[:, 1], AF.Exp,
                                bias=nbias_cols[2 * p + 1][:])
                        if pend_att is not None:
                            emit_pvlb(*pend_att)
                        pend_att = (g, ptile)
                    emit_pvlb(*pend_att)

                    rl = att_sb.tile([128, 2, QB], F32, tag="rl",
                                     name=f"rl{p}_{qb}")
                    nc.vector.reciprocal_approx_fast(out=rl[0:65, 0, :],
                                                     in_=pv[0:65, 0, :])
                    nc.vector.reciprocal_approx_fast(out=rl[0:65, 1, :],
                                                     in_=pv[0:65, 1, :])
                    rld = rl_sp[qb % 2]
                    nc.gpsimd.dma_start(out=rld, in_=rl[64:65, :, :])
                    rlbc = att_sb.tile([64, 2, QB], F32, tag="rlbc",
                                       name=f"rlbc{p}_{qb}")
                    for h in (0, 1):
                        nc.gpsimd.dma_start(
                            out=rlbc[:, h, :],
                            in_=rld[0:1, h * QB:(h + 1) * QB].to_broadcast(
                                (64, QB)))
                    for h in (0, 1):
                        of = ofin_pool.tile([64, QB], BF16,
                                            tag=f"of{p}_{h}_{qb}",
                                            name=f"of{p}_{h}_{qb}")
                        nc.vector.tensor_mul(of, pv[0:64, h, :],
                                             rlbc[:, h, :])
                        o_fin[(p, h, qb)] = of
                    if qb_done is not None:
                        qb_done(qb)

            # ================= out projection (pipelined evac) ============
            pend_o = None
            outT_v = out_d.ap().rearrange("(cb p) t -> p cb t", p=128)

            def flush_o():
                nonlocal pend_o
                if pend_o is None:
                    return
                qb, cb, op, ot = pend_o
                nc.vector.tensor_scalar_add(ot[:, cb, :], op, wob_cols[cb])
                pend_o = None

            def emit_outproj_qb(qb):
                nonlocal pend_o
                ot = ot_pool.tile([128, CT, QB], BF16, tag="ot",
                                  name=f"ot{qb}")
                for cb in range(CT):
                    op = ps_mm.tile([128, QB], F32, tag="mm",
                                    name=f"op{qb}_{cb}")
                    for ph in range(2 * PAIRS):
                        nc.tensor.matmul(op,
                                         wo_res[:, ph,
                                                cb * 128:(cb + 1) * 128],
                                         o_fin[(ph // 2, ph % 2, qb)][:],
                                         start=(ph == 0),
                                         stop=(ph == 2 * PAIRS - 1))
                    flush_o()
                    pend_o = (qb, cb, op, ot)
                flush_o()
                nc.gpsimd.dma_start(
                    out=outT_v[:, :, qb * QB:(qb + 1) * QB], in_=ot)

            # ======== interleaved pair-level schedule ========
            for p in range(PAIRS):
                emit_proj_pair(p)
            flush_qk()
            for p in range(PAIRS - 1):
                emit_att_pair(p)
            emit_att_pair(PAIRS - 1, qb_done=emit_outproj_qb)
            flush_o()

    nc.compile()
    return nc


# ======================= host-side sharding =======================

def shard_inputs(x, w_qkv, b_qkv, w_out, logit_scale):
    x = np.ascontiguousarray(np.asarray(x, dtype=np.float32))
    w_qkv = np.asarray(w_qkv, dtype=np.float32)
    b_qkv = np.asarray(b_qkv, dtype=np.float32)
    w_out = np.asarray(w_out, dtype=np.float32)
    ls = np.asarray(logit_scale, dtype=np.float32).reshape(-1)
    s_all = np.exp(np.minimum(ls, LOG100)).astype(np.float32)

    Wq = w_qkv[0 * INNER:1 * INNER]
    Wk = w_qkv[1 * INNER:2 * INNER]
    Wv = w_qkv[2 * INNER:3 * INNER]
    bq = b_qkv[0 * INNER:1 * INNER]
    bk = b_qkv[1 * INNER:2 * INNER]
    bv = b_qkv[2 * INNER:3 * INNER]

    xT = [np.ascontiguousarray(x[b].T.astype(ml_bf16)) for b in range(B)]

    per_hg = {}
    merge = [True] * 4
    for hg in range(2):
        heads = list(range(hg * 8, hg * 8 + 8))
        rows, brows = [], []
        for p in range(4):
            g0, g1 = heads[2 * p], heads[2 * p + 1]
            if s_all[g0] != s_all[g1]:
                merge[p] = False
            rows += [Wq[g0 * 64:(g0 + 1) * 64], Wq[g1 * 64:(g1 + 1) * 64],
                     Wk[g0 * 64:(g0 + 1) * 64], Wk[g1 * 64:(g1 + 1) * 64]]
            brows += [bq[g0 * 64:(g0 + 1) * 64], bq[g1 * 64:(g1 + 1) * 64],
                      bk[g0 * 64:(g0 + 1) * 64], bk[g1 * 64:(g1 + 1) * 64]]
        wqkT = np.ascontiguousarray(
            np.concatenate(rows, axis=0).T.astype(ml_bf16))
        bqk = np.ascontiguousarray(
            np.concatenate(brows, axis=0)).reshape(8, 128, 1)
        vsl = slice(hg * 512, (hg + 1) * 512)
        wvT = np.ascontiguousarray(Wv[vsl].T.astype(ml_bf16))
        # [64, (2p+h)*C]: head-half major out-proj weights for K=64 matmuls
        woT = np.ascontiguousarray(
            w_out[:, vsl].reshape(D_MODEL, 4, 2, 64).transpose(3, 1, 2, 0)
            .reshape(64, 8 * D_MODEL).astype(ml_bf16))
        wob = np.ascontiguousarray(
            (w_out[:, vsl] @ bv[vsl]).astype(np.float32)).reshape(8, 128, 1)
        scl = np.stack([-s_all[heads], np.log(s_all[heads])],
                       axis=1).astype(np.float32).reshape(8, 2, 1)
        per_hg[hg] = dict(wqkT=wqkT, bqk=bqk, wvT=wvT, woT=woT, wob=wob,
                          scl=scl)

    sel2 = np.zeros((2, 128), dtype=np.float32)
    sel2[0, 0:64] = 1.0
    sel2[1, 64:128] = 1.0
    sel2T = np.ascontiguousarray(sel2.T)
    onec = np.ones((1, 1), dtype=ml_bf16)
    in_maps = []
    for c in range(N_CORES):
        b, hg = c // 2, c % 2
        m = dict(per_hg[hg])
        m["xT"] = xT[b]
        m["sel2T"] = sel2T
        m["onec"] = onec
        in_maps.append(m)
    return in_maps, tuple(merge)


_NC_CACHE = {}
TRACE = False
LAST_RESULT = None


def kernel(x, w_qkv, b_qkv, w_out, logit_scale):
    global LAST_RESULT
    in_maps, merge_pairs = shard_inputs(x, w_qkv, b_qkv, w_out, logit_scale)
    cfg = Cfg(merge_pairs=merge_pairs)
    if merge_pairs not in _NC_CACHE:
        _NC_CACHE[merge_pairs] = build(cfg)
    nc = _NC_CACHE[merge_pairs]
    res = run_bass_kernel_spmd(nc, in_maps, core_ids=list(range(N_CORES)),
                               trace=TRACE)
    LAST_RESULT = res
    outs = [np.asarray(res.results[c]["out"], dtype=np.float32)
            for c in range(N_CORES)]
    full = np.empty((B, N, D_MODEL), dtype=np.float32)
    for b in range(B):
        full[b] = (outs[2 * b] + outs[2 * b + 1]).T
    return full

